# revision 15
# baseline (speedup 1.0000x reference)
"""Distributed multi-head attention (RoPE, non-causal) on 8 TRN2 NeuronCores.

Sharding: tensor-parallel over heads. Core c owns heads {2c, 2c+1}:
  - wq/wk/wv rows c*256:(c+1)*256 (output dim), x replicated (pre-transposed),
  - attention computed locally per (batch, head),
  - AllToAll redistributes attention outputs: peer p receives my heads'
    columns for ITS seq-slice (q-cols {qc*512 + p*64 + j}) -- 8x less wire
    traffic than the v2 AllGather,
  - each core then computes ALL 2048 output dims for its own seq-slice,
    streaming the full woT from DRAM per batch (it no longer fits in SBUF).
Host side only shards/casts inputs and re-permutes the per-core seq-slices
into the full output -- all FLOPs run on device.

v3 schedule (evolved from v2's interleaved emission):
  - One globally interleaved stream per batch window: attention(b) in
    4-matmul steps with proj(b+1) and wo(b-1) chains paced between.
  - AllToAll per batch-half: A2A(b, h0) fires mid-window(b) (after units
    0-3), A2A(b, h1) at window(b+1) start.  Each moves only 512KB.
  - wo(b) runs in window(b+1): 16 oc-chains of N=256 against the gathered
    [2048, 256] slice; woT streamed per-oc ([128,16,128] tiles, ring of 4).
  - batch 3 wo split by half: h0-chains late in window(3), h1-chains in the
    tail (reusing the last 4 resident woT tiles first to cut the re-stream).
  - Startup: PE warm-up transposes ramp the clock while piecewise weight/x
    DMAs land; first real matmul needs only 768KB (vs 3MB in v2).

Layout/precision (as v2): all matmuls bf16, PSUM f32, RoPE halves via host
row-permute of wq/wk, transposed scores + ones-column denominator, exp on
[128,1024] PSUM pairs, transposes deferred one step.
"""

import numpy as np
import ml_dtypes

B, S, D, H = 4, 2048, 2048, 16
HD = 128            # head dim
NCORES = 8
HPC = H // NCORES   # heads per core = 2
OSL = HPC * HD      # per-core o-slice = 256
ROWS = B * S        # 8192 flattened rows
DCH = D // 128      # 16 contraction chunks
SCH = 512           # seq chunk for projections
KB = S // 128       # 16 k-blocks per batch
QC = 512            # q chunk in attention
NQC = S // QC       # 4
QW = QC // NCORES   # 64: per-peer q-cols per quarter
XP = 4              # x fetch pieces
XCPT = DCH // XP    # 4 contraction chunks per x piece
WARMUP = 40         # PE clock-ramp transposes at t=0

BF16 = ml_dtypes.bfloat16
_NC_CACHE = None


def _build():
    import concourse.bass as bass  # noqa: F401
    import concourse.mybir as mybir
    import concourse.tile as tile
    from concourse import bacc
    from concourse.masks import make_identity

    fp32 = mybir.dt.float32
    bf16 = mybir.dt.bfloat16

    nc = bacc.Bacc(
        "TRN2",
        target_bir_lowering=False,
        debug=False,
        num_devices=NCORES,
    )

    xT = nc.declare_dram_parameter("xT", [D, ROWS], bf16, isOutput=False)
    wqT = nc.declare_dram_parameter("wqT", [D, OSL], bf16, isOutput=False)
    wkT = nc.declare_dram_parameter("wkT", [D, OSL], bf16, isOutput=False)
    wvT = nc.declare_dram_parameter("wvT", [D, OSL], bf16, isOutput=False)
    woT = nc.declare_dram_parameter("woT", [D, D], bf16, isOutput=False)
    cosd = nc.declare_dram_parameter("cosd", [128, S], fp32, isOutput=False)
    sind = nc.declare_dram_parameter("sind", [128, S], fp32, isOutput=False)
    outp = nc.declare_dram_parameter("out", [D, B * OSL], bf16, isOutput=True)

    inv_sqrt_hd = 1.0 / float(np.sqrt(HD))

    with tile.TileContext(nc) as tc:
        with (
            tc.tile_pool(name="glob", bufs=1) as glob,
            tc.tile_pool(name="dram", bufs=1, space="DRAM") as dram,
            tc.tile_pool(name="qkv", bufs=2) as qkv,
            tc.tile_pool(name="xtp", bufs=7) as xtp,
            tc.tile_pool(name="attp", bufs=4) as attp,
            tc.tile_pool(name="wop", bufs=2) as wop,
            tc.tile_pool(name="ghp", bufs=2) as ghp,
            tc.tile_pool(name="tmpp", bufs=1) as tmpp,
            tc.tile_pool(name="smalls", bufs=2) as smalls,
            tc.tile_pool(name="atp", bufs=2) as atp,
            tc.tile_pool(name="otp", bufs=2) as otp,
            tc.tile_pool(name="psA", bufs=2, space="PSUM") as psA,
            tc.tile_pool(name="psB", bufs=2, space="PSUM") as psB,
            tc.tile_pool(name="psCD", bufs=2, space="PSUM") as psCD,
        ):
            ident = glob.tile([128, 128], bf16, name="ident")

            # weights as piece-tile lists: (tiles, chunks-per-tile)
            def walloc(nm, cpt):
                n = DCH // cpt
                return ([glob.tile([128, cpt, OSL], bf16, name=f"{nm}{i}")
                         for i in range(n)], cpt)

            wk_t = walloc("wk", 4)   # 4 pieces so the first chain starts early
            wq_t = walloc("wq", 8)
            wv_t = walloc("wv", 8)
            cosb = glob.tile([128, S], fp32, name="cosb")
            sinb = glob.tile([128, S], fp32, name="sinb")

            def wsl(w, c, cols):
                tiles, cpt = w
                return tiles[c // cpt][:, c % cpt, cols]

            def wdma_piece(w, src, i):
                tiles, cpt = w
                nc.gpsimd.dma_start(
                    tiles[i][:],
                    src[i * cpt * 128:(i + 1) * cpt * 128, :]
                    .rearrange("(c p) n -> p c n", p=128))

            xstate = {}

            def fetch_x(b, sc):
                if (b, sc) in xstate:
                    return
                col0 = b * S + sc * SCH
                xts = []
                for i in range(XP):
                    xt_ = xtp.tile([128, XCPT, SCH], bf16, name=f"xt{i}",
                                   tag="xt")
                    nc.gpsimd.dma_start(
                        xt_[:],
                        xT[i * XCPT * 128:(i + 1) * XCPT * 128,
                           col0:col0 + SCH]
                        .rearrange("(c p) n -> p c n", p=128))
                    xts.append(xt_)
                xstate[(b, sc)] = xts

            def xsl(b, sc, c, cols):
                return xstate[(b, sc)][c // XCPT][:, c % XCPT, cols]

            # AllToAll buffers, one pair per (batch, half)
            a2a_in = {}
            a2a_out = {}
            for b in range(B):
                for hh in range(2):
                    a2a_in[(b, hh)] = dram.tile(
                        [NCORES, OSL, 2 * QW], bf16, name=f"ain{b}_{hh}")
                    a2a_out[(b, hh)] = dram.tile(
                        [NCORES, OSL, 2 * QW], bf16, name=f"aout{b}_{hh}")

            def emit_a2a(b, hh):
                nc.gpsimd.collective_compute(
                    "AllToAll",
                    mybir.AluOpType.bypass,
                    ins=[a2a_in[(b, hh)].opt()],
                    outs=[a2a_out[(b, hh)].opt()],
                    replica_groups=[list(range(NCORES))],
                )

            # ---------- projection chains ----------
            qkvstate = {}

            def get_qkv(b):
                if b not in qkvstate:
                    qt = qkv.tile([128, HPC, S], bf16, name="qt", tag="qt")
                    kt = qkv.tile([128, HPC, S], bf16, name="kt", tag="kt")
                    vt = qkv.tile([128, KB, HPC, HD + 1], bf16, name="vt",
                                  tag="vt")
                    qkvstate[b] = (qt, kt, vt)
                return qkvstate[b]

            def qk_chain(b, sc, w, dst_idx, h):
                qt, kt, vt = get_qkv(b)
                dstT = (qt, kt)[dst_idx]
                cosr = cosb[:, sc * SCH:(sc + 1) * SCH]
                sinr = sinb[:, sc * SCH:(sc + 1) * SCH]
                ps = psA.tile([128, SCH], fp32, name="ps_proj", tag="psA")
                for c in range(DCH):
                    nc.tensor.matmul(
                        ps[:],
                        wsl(w, c, slice(h * HD, (h + 1) * HD)),
                        xsl(b, sc, c, slice(None)),
                        start=(c == 0), stop=(c == DCH - 1))
                m1 = tmpp.tile([128, SCH], fp32, name="m1", tag="m1")
                m2 = tmpp.tile([128, SCH], fp32, name="m2", tag="m2")
                nc.vector.tensor_mul(m1[:], ps[:], cosr)
                nc.vector.tensor_mul(
                    m2[0:64, :], ps[64:128, :], sinr[0:64, :])
                nc.vector.tensor_mul(
                    m2[64:128, :], ps[0:64, :], sinr[64:128, :])
                sl = slice(sc * SCH, (sc + 1) * SCH)
                nc.vector.tensor_sub(
                    dstT[0:64, h, sl], m1[0:64, :], m2[0:64, :])
                nc.vector.tensor_add(
                    dstT[64:128, h, sl], m2[64:128, :], m1[64:128, :])

            vt_init = set()

            def v_chain(b, sc, ssb):
                qt, kt, vt = get_qkv(b)
                if b not in vt_init:
                    vt_init.add(b)
                    nc.vector.memset(vt[:, :, :, HD:HD + 1], 1.0)
                kb = sc * (SCH // 128) + ssb
                psv = psA.tile([128, OSL], fp32, name="psv", tag="psA")
                for c in range(DCH):
                    nc.tensor.matmul(
                        psv[:],
                        xsl(b, sc, c, slice(ssb * 128, (ssb + 1) * 128)),
                        wsl(wv_t, c, slice(None)),
                        start=(c == 0), stop=(c == DCH - 1))
                nc.vector.tensor_copy(
                    vt[:, kb, :, 0:HD],
                    psv[:].rearrange("p (h d) -> p h d", h=HPC))

            def proj_fillers(b):
                """Closure list emitting proj(b): fetches + chains."""
                items = []
                for sc in range(S // SCH):
                    if (b, sc) not in xstate:
                        items.append((0, lambda b=b, sc=sc: fetch_x(b, sc)))
                    for h in range(HPC):
                        items.append(
                            (0, lambda b=b, sc=sc, h=h:
                             qk_chain(b, sc, wk_t, 1, h)))
                    for h in range(HPC):
                        items.append(
                            (0, lambda b=b, sc=sc, h=h:
                             qk_chain(b, sc, wq_t, 0, h)))
                    if sc + 1 < S // SCH:
                        if (b, sc + 1) not in xstate:
                            items.append(
                                (0, lambda b=b, sc=sc: fetch_x(b, sc + 1)))
                    elif b + 1 < B:
                        items.append((0, lambda b=b: fetch_x(b + 1, 0)))
                    for ssb in range(SCH // 128):
                        items.append(
                            (0, lambda b=b, sc=sc, ssb=ssb:
                             v_chain(b, sc, ssb)))
                return items

            # ---------- attention ----------
            expstate = {}
            atstate = {}

            def score_pair(b, u, j):
                """Two scores matmuls into one 2-bank PSUM tile + one exp."""
                qt, kt, vt = get_qkv(b)
                h, qc = u % HPC, u // HPC
                eh, jj = j // 4, j % 4
                if (u % 2, eh) not in expstate or \
                        expstate[(u % 2, eh)][0] != (b, u):
                    t = attp.tile([128, KB // 2, QC], bf16, name="expT",
                                  tag="expT")
                    expstate[(u % 2, eh)] = ((b, u), t)
                expT = expstate[(u % 2, eh)][1]
                pss = psB.tile([128, 2 * QC], fp32, name="pss", tag="psB")
                for i in range(2):
                    kb = eh * (KB // 2) + 2 * jj + i
                    nc.tensor.matmul(
                        pss[:, i * QC:(i + 1) * QC],
                        kt[:, h, kb * 128:(kb + 1) * 128],
                        qt[:, h, qc * QC:(qc + 1) * QC],
                        start=True, stop=True)
                nc.scalar.activation(
                    expT[:, 2 * jj:2 * jj + 2, :],
                    pss[:].rearrange("p (a n) -> p a n", a=2),
                    mybir.ActivationFunctionType.Exp,
                    scale=inv_sqrt_hd)

            def attnv_mms(b, u, s):
                """16 accumulating matmuls for q-subblock s of unit u."""
                qt, kt, vt = get_qkv(b)
                h = u % HPC
                e0 = expstate[(u % 2, 0)][1]
                e1 = expstate[(u % 2, 1)][1]
                if (b, u) not in atstate:
                    atstate[(b, u)] = atp.tile([128, QC], bf16, name="a_t",
                                               tag="a_t")
                a_t = atstate[(b, u)]
                pso = psCD.tile([128, HD + 1], fp32, name="pso", tag="psCD")
                for kb in range(KB):
                    eT = e0 if kb < KB // 2 else e1
                    nc.tensor.matmul(
                        pso[:],
                        eT[:, kb % (KB // 2), s * 128:(s + 1) * 128],
                        vt[:, kb, h, :],
                        start=(kb == 0), stop=(kb == KB - 1))
                return pso, a_t

            def finish(b, u, s, pso, a_t):
                """normalize + transpose + copy (+ A2A-input DMA)."""
                h, qc = u % HPC, u // HPC
                rc = smalls.tile([128, 1], fp32, name="rc", tag="rc")
                nc.vector.reciprocal(rc[:], pso[:, HD:HD + 1])
                a_sb = smalls.tile([128, HD], bf16, name="a_sb", tag="a_sb")
                nc.vector.tensor_scalar_mul(a_sb[:], pso[:, 0:HD], rc[:])
                pst = psCD.tile([128, 128], bf16, name="pst", tag="psCD")
                nc.tensor.transpose(pst[:], a_sb[:], ident[:])
                nc.vector.tensor_copy(a_t[:, s * 128:(s + 1) * 128], pst[:])
                if s == 3:
                    dst = a2a_in[(b, qc // 2)]
                    q0 = (qc % 2) * QW
                    for p in range(NCORES):
                        nc.sync.dma_start(
                            dst[p, h * HD:(h + 1) * HD, q0:q0 + QW],
                            a_t[:, p * QW:(p + 1) * QW])

            # ---------- wo stage (q-sharded, woT streamed) ----------
            wostate = {}
            ghstate = {}
            WOCT = 4          # output-dim chunks per weight tile (2MB DMAs)

            def wo_w(b, t):
                tl = wop.tile([128, DCH, WOCT * 128], bf16, name="wot",
                              tag="wot")
                nc.scalar.dma_start(
                    tl[:],
                    woT[:, t * WOCT * 128:(t + 1) * WOCT * 128]
                    .rearrange("(c p) n -> p c n", p=128))
                wostate[(b, t)] = tl

            def gather_full(b):
                gh = ghp.tile([128, DCH, 2 * 2 * QW], bf16, name="gh",
                              tag="gh")
                for hh in range(2):
                    nc.sync.dma_start(
                        gh[:, :, hh * 2 * QW:(hh + 1) * 2 * QW],
                        a2a_out[(b, hh)]
                        .rearrange("k (c p) n -> p (k c) n", p=128))
                ghstate[b] = gh

            def gather_half(b, hh):
                gh = ghp.tile([128, DCH, 2 * QW], bf16, name="ghh", tag="gh")
                nc.sync.dma_start(
                    gh[:],
                    a2a_out[(b, hh)]
                    .rearrange("k (c p) n -> p (k c) n", p=128))
                ghstate[(b, hh)] = gh

            def wo_chain(b, oc, gh, col0, ncols):
                """psw[oc-block, ncols] = sum_c woT[c, oc].T @ gh[c]."""
                wt = wostate[(b, oc // WOCT)]
                o0 = (oc % WOCT) * 128
                psw = psA.tile([128, ncols], fp32, name="psw", tag="psA")
                for c in range(DCH):
                    nc.tensor.matmul(
                        psw[:], wt[:, c, o0:o0 + 128], gh[:, c, :],
                        start=(c == 0), stop=(c == DCH - 1))
                out_t = otp.tile([128, ncols], bf16, name="out_t", tag="out_t")
                nc.vector.tensor_copy(out_t[:], psw[:])
                nc.sync.dma_start(
                    outp[oc * 128:(oc + 1) * 128, col0:col0 + ncols],
                    out_t[:])

            def wo_fillers(b, tiles):
                """wo(b) chains for the given weight-tile indices.  In the
                short window(3) (b==2) the A2A peer-skew can reach ~30us, so
                gate the gather/chains later to keep attention matmuls ahead
                of them in the in-order PE queue."""
                g = 22 if b == 2 else 8
                gc = 24 if b == 2 else 10
                items = [(0, lambda b=b: wo_w(b, tiles[0]))]
                if len(tiles) > 1:
                    items.append((2, lambda b=b: wo_w(b, tiles[1])))
                items.append((g, lambda b=b: gather_full(b)))
                for i, t in enumerate(tiles):
                    for oc in range(t * WOCT, (t + 1) * WOCT):
                        items.append(
                            (gc + i, lambda b=b, oc=oc: wo_chain(
                                b, oc, ghstate[b], b * OSL, 256)))
                    if i + 2 < len(tiles):
                        items.append(
                            (gc + i,
                             lambda b=b, t2=tiles[i + 2]: wo_w(b, t2)))
                if 2 not in tiles:
                    # deferred tiles: stream now so window(b+2) can chain on
                    # them at gate 0 (dependency-free A2A-latency cover)
                    items.append((15, lambda b=b: wo_w(b, 2)))
                    items.append((33, lambda b=b: wo_w(b, 3)))
                return items

            def wo_deferred_fillers(b):
                """wo(b) tiles 2,3 chains, deferred to window(b+2): weight
                tiles were pre-streamed in window(b+1), data ready since
                window(b+1) start -- pure gate-0 filler."""
                items = []
                for oc in range(2 * WOCT, 4 * WOCT):
                    items.append(
                        (0, lambda b=b, oc=oc: wo_chain(
                            b, oc, ghstate[b], b * OSL, 256)))
                return items

            def wo3_h0_fillers():
                """wo(3) first half, late in window(3).  Tile order 2,3,0,1
                leaves tiles 0,1 resident in the ring for the tail."""
                items = [(20, lambda: wo_w(3, 2)),
                         (21, lambda: wo_w(3, 3)),
                         (26, lambda: gather_half(3, 0))]
                for oc in range(8, 16):
                    items.append(
                        (28, lambda oc=oc: wo_chain(
                            3, oc, ghstate[(3, 0)], 3 * OSL, 128)))
                items.append((29, lambda: wo_w(3, 0)))
                items.append((30, lambda: wo_w(3, 1)))
                return items

            # ---------- the interleaved window ----------
            pending = [None]

            def flush_pending():
                if pending[0] is not None:
                    fin = pending[0]
                    pending[0] = None
                    fin()

            def window(b):
                fillers = []
                if b >= 2:
                    fillers.extend(wo_deferred_fillers(b - 2))
                if b + 1 < B:
                    fillers.extend(proj_fillers(b + 1))
                if b >= 1:
                    tiles = [0, 1, 2, 3] if b == B - 1 else [0, 1]
                    fillers.extend(wo_fillers(b - 1, tiles))
                if b == B - 1:
                    fillers.extend(wo3_h0_fillers())
                nf = len(fillers)
                nsteps = 9 * 4
                emitted = [0]

                def pace(step):
                    target = (nf * (step + 1) + nsteps - 1) // nsteps
                    while (emitted[0] < min(target, nf)
                           and fillers[emitted[0]][0] <= step):
                        fillers[emitted[0]][1]()
                        emitted[0] += 1

                step = 0
                for u in range(9):
                    for s in range(4):
                        if u < 8:
                            score_pair(b, u, 2 * s)
                            score_pair(b, u, 2 * s + 1)
                        if u >= 1:
                            pso, a_t = attnv_mms(b, u - 1, s)
                            flush_pending()
                            pending[0] = (
                                lambda b=b, u=u - 1, s=s, pso=pso, a_t=a_t:
                                finish(b, u, s, pso, a_t))
                            if u == 5 and s == 0:
                                emit_a2a(b, 0)  # units 0-3 finished
                        elif u == 0 and s == 0:
                            # previous window's last finish, after 2 pairs
                            flush_pending()
                            if b >= 1:
                                emit_a2a(b - 1, 1)
                        pace(step)
                        step += 1

            # ---------- emission ----------
            make_identity(nc, ident[:])
            warm = psCD.tile([128, 128], bf16, name="warm", tag="psCD")
            for _ in range(WARMUP):
                nc.tensor.transpose(warm[:], ident[:], ident[:])

            # startup DMAs: what the first chain needs, first, split
            # across extra queues for a bigger early-bandwidth share
            for hf in range(2):
                nc.gpsimd.dma_start(
                    wk_t[0][0][:, 2 * hf:2 * hf + 2, :],
                    wkT[hf * 256:(hf + 1) * 256, :]
                    .rearrange("(c p) n -> p c n", p=128))
            xts00 = []
            for i in range(XP):
                xt_ = xtp.tile([128, XCPT, SCH], bf16, name=f"xt{i}",
                               tag="xt")
                for hf in range(2):
                    nc.gpsimd.dma_start(
                        xt_[:, 2 * hf:2 * hf + 2, :],
                        xT[(i * XCPT + 2 * hf) * 128:
                           (i * XCPT + 2 * hf + 2) * 128, 0:SCH]
                        .rearrange("(c p) n -> p c n", p=128))
                xts00.append(xt_)
            xstate[(0, 0)] = xts00
            nc.gpsimd.dma_start(cosb[:, 0:SCH], cosd[:, 0:SCH])
            nc.gpsimd.dma_start(sinb[:, 0:SCH], sind[:, 0:SCH])
            for i in (1, 2, 3):
                wdma_piece(wk_t, wkT, i)
            wdma_piece(wq_t, wqT, 0)
            wdma_piece(wq_t, wqT, 1)
            wdma_piece(wv_t, wvT, 0)
            wdma_piece(wv_t, wvT, 1)
            fetch_x(0, 1)
            nc.gpsimd.dma_start(cosb[:, SCH:], cosd[:, SCH:])
            nc.gpsimd.dma_start(sinb[:, SCH:], sind[:, SCH:])

            for _, it in proj_fillers(0):
                it()
            for b in range(B):
                window(b)

            # ---------- tail: finish batch 3 ----------
            flush_pending()            # finish(3, 7, 3) + its A2A-in write
            emit_a2a(3, 1)
            gather_half(3, 1)          # issues now, fires when the CC lands
            for oc in range(0, 8):     # held-back h0 chains cover the A2A
                wo_chain(3, oc, ghstate[(3, 0)], 3 * OSL, 128)
            for oc in range(0, 8):     # tiles 0,1 still resident
                wo_chain(3, oc, ghstate[(3, 1)], 3 * OSL + 128, 128)
            wo_w(3, 2)                 # overlaps chains above
            for oc in range(8, 12):
                wo_chain(3, oc, ghstate[(3, 1)], 3 * OSL + 128, 128)
            wo_w(3, 3)
            for oc in range(12, 16):
                wo_chain(3, oc, ghstate[(3, 1)], 3 * OSL + 128, 128)

    nc.compile()
    return nc


def _shard_inputs(x, freqs_cos, freqs_sin, wq, wk, wv, wo):
    xf = np.asarray(x, dtype=np.float32).reshape(ROWS, D)
    xT = np.ascontiguousarray(xf.T).astype(BF16)
    fcT = np.asarray(freqs_cos, dtype=np.float32).T  # [64, S]
    fsT = np.asarray(freqs_sin, dtype=np.float32).T
    cosd = np.ascontiguousarray(np.concatenate([fcT, fcT], 0))  # [128, S]
    sind = np.ascontiguousarray(np.concatenate([fsT, fsT], 0))
    # even indices (real half) then odd (imag half), per head
    perm = np.concatenate([np.arange(0, HD, 2), np.arange(1, HD, 2)])
    woT_full = np.ascontiguousarray(
        np.asarray(wo, dtype=np.float32).T).astype(BF16)  # [in d, out]
    in_maps = []
    for c in range(NCORES):
        rows = slice(c * OSL, (c + 1) * OSL)
        wq_c = np.asarray(wq)[rows].reshape(HPC, HD, D)[:, perm, :] \
            .reshape(OSL, D)
        wk_c = np.asarray(wk)[rows].reshape(HPC, HD, D)[:, perm, :] \
            .reshape(OSL, D)
        in_maps.append({
            "xT": xT,
            "wqT": np.ascontiguousarray(wq_c.T).astype(BF16),
            "wkT": np.ascontiguousarray(wk_c.T).astype(BF16),
            "wvT": np.ascontiguousarray(np.asarray(wv)[rows].T).astype(BF16),
            "woT": woT_full,
            "cosd": cosd,
            "sind": sind,
        })
    return in_maps


def run(inputs, trace=False, trace_cores=None):
    """Build (cached), run on 8 cores; returns (full_output, results)."""
    global _NC_CACHE
    from concourse.bass_utils import run_bass_kernel_spmd
    if _NC_CACHE is None:
        _NC_CACHE = _build()
    in_maps = _shard_inputs(**inputs)
    res = run_bass_kernel_spmd(
        _NC_CACHE, in_maps, core_ids=list(range(NCORES)), trace=trace,
        trace_cores=trace_cores)
    parts = [np.asarray(res.results[c]["out"], dtype=np.float32)
             for c in range(NCORES)]                    # each [D, B*OSL]
    stack = np.stack(parts, 0).reshape(NCORES, D, B, NQC, QW)
    # full[b, qc*512 + c*64 + j, d] = stack[c, d, b, qc, j]
    full = stack.transpose(2, 3, 0, 4, 1).reshape(B, S, D)
    return np.ascontiguousarray(full), res


def kernel(x, freqs_cos, freqs_sin, wq, wk, wv, wo):
    full, _ = run(dict(x=x, freqs_cos=freqs_cos, freqs_sin=freqs_sin,
                       wq=wq, wk=wk, wv=wv, wo=wo))
    return full
